# revision 5
# baseline (speedup 1.0000x reference)
# Trainium2 Bass kernel: X-stationary formulation for the LeNet-C3
# sparse-connection conv problem.
#
# Math: VALID 2D conv, input [32, 512, 512, 6] f32, dense kernel [5,5,6,16]
# (assembled from the sparse C3 tables), + bias -> [32, 508, 508, 16] f32.
#
# Formulation (v2): host pre-tiles the input as x[img, xtile, 128, 516] bf16
# where the 128 partition rows are (x-position, channel)-flat columns of a
# 21-px window (126 rows) + a constant-ones row (126) + a zero row (127),
# and the free elements are image rows y. The matmul uses the INPUT as
# stationary: psum[y, (px,co)] += xt[:, y0+dy:y0+dy+128].T @ wband_dy, with
# wband_dy a host-built banded [128, 272] moving matrix (17 out px x 16 out
# ch), 5 accumulating matmuls per (xtile, yblock). The ones-row of x times a
# bias row in wband_0 folds the bias add into the matmul. PSUM holds
# [y (partitions), (px,co) (free)] so output DMAs are contiguous 1088B runs.
#
# v3 scheduling fixes over v2 (which stalled the PE ~3.5us per x-tile and
# kept it HAM-throttled at 1.2 GHz 59% of the time):
#   - y padded to 512 on the device output (host crops to 508) so all 4
#     y-blocks are uniform 128 partitions; every stationary is full 128
#     columns (FWL-eligible).
#   - ONE merged output DMA per x-tile ([128, 4, npx*16] AP) issued on the
#     scalar queue; the sync queue carries ONLY input prefetch loads, so
#     output DMAs waiting on copies can never head-of-line-block prefetch.
#   - all PSUM->SBUF copies on the vector engine.

import numpy as np
import ml_dtypes

BATCH, H, W, CIN, COUT, FS = 32, 512, 512, 6, 16, 5
N_CORES = 8
IMGS_PER_CORE = BATCH // N_CORES  # 4
HO = WO = H - FS + 1  # 508
FLAT = W * CIN  # 3072
HP = 516                    # padded y extent of the device x tiles
HOP = 512                   # padded y extent of the device output
PXT = 17                    # output pixels per x-tile
XTILES = 30                 # 17*29 + 15 = 508
KCOLS = (PXT + 4) * CIN     # 126 data rows of the stationary
KPAD = 128                  # + ones row (126) + zero row (127)
XSTRIDE = PXT * CIN         # 102 flat columns per x-tile step
NMOV = PXT * COUT           # 272 moving columns
YB = (0, 128, 256, 384)     # uniform 128-row y-blocks

_CACHE = {}


def _dense_kernel_np(weights3, weights4, weights4_4, weights6):
    """Numpy port of reference._dense_kernel: [5,5,6,16] dense conv kernel."""
    f = weights3.shape[0]
    Wd = np.zeros((f, f, CIN, COUT), dtype=np.float32)
    for i in range(6):
        for m in range(3):
            Wd[:, :, (i + m) % 6, i] = weights3[:, :, m, i]
    for k in range(6):
        for m in range(4):
            Wd[:, :, (k + m) % 6, 6 + k] = weights4[:, :, m, k]
    for k in range(3):
        for m, off in enumerate((0, 1, 3, 4)):
            Wd[:, :, (k + off) % 6, 12 + k] = weights4_4[:, :, m, k]
    Wd[:, :, :, 15] = weights6[:, :, :, 0]
    return Wd


def _build_wband(Wd, bias1):
    """[KPAD, FS, NMOV]: wband[xa*6+ci, dy, j*16+co] = Wd[dy, xa-j, ci, co]
    for 0 <= xa-j < 5; row 126 of dy=0 carries the bias."""
    wb = np.zeros((KPAD, FS, NMOV), dtype=np.float32)
    for dy in range(FS):
        for j in range(PXT):
            for dxa in range(FS):
                xa = j + dxa
                for ci in range(CIN):
                    wb[xa * CIN + ci, dy, j * COUT:(j + 1) * COUT] = \
                        Wd[dy, dxa, ci, :]
    wb[KCOLS, 0, :] = np.tile(np.asarray(bias1, np.float32), PXT)
    return wb


def _split_excess_waits(nc, max_waits=1):
    """This image's walrus rejects instructions carrying more than one sem
    wait ("Too many sync wait commands" in setupSyncWait). Tile freely
    attaches several waits to one instruction. Hoist the extras onto
    nofuse NOPs inserted just before, on the same engine — identical
    semantics (all waits retired before the instruction issues)."""
    import concourse.mybir as mybir

    for f in nc.m.functions:
        for bb in f.blocks:
            new_list = []
            changed = False
            for inst in bb.instructions:
                si = inst.sync_info
                waits = list(si.on_wait) if si and si.on_wait else []
                if len(waits) > max_waits:
                    changed = True
                    for k, w in enumerate(waits[max_waits:]):
                        nop = mybir.InstNoOp(
                            name=f"{inst.name}-wsplit{k}",
                            sync_info=mybir.SyncInfo(on_wait=[w], on_update=[]),
                            bass_nofuse=True,
                            engine=inst.engine,
                        )
                        new_list.append(nop)
                    si.on_wait = waits[:max_waits]
                new_list.append(inst)
            if changed:
                bb.instructions = new_list
    return nc


def _build_nc(n_imgs=IMGS_PER_CORE):
    import concourse.bass as bass
    import concourse.mybir as mybir
    from concourse.tile import TileContext

    nc = bass.Bass(trn_type="TRN2")
    x = nc.dram_tensor("x", (n_imgs, XTILES, KPAD, HP), mybir.dt.bfloat16,
                       kind="ExternalInput")
    w = nc.dram_tensor("w", (KPAD, FS * NMOV), mybir.dt.bfloat16,
                       kind="ExternalInput")
    out = nc.dram_tensor("out", (n_imgs, HOP, WO, COUT), mybir.dt.float32,
                         kind="ExternalOutput")

    with TileContext(nc) as tc:
        with tc.tile_pool(name="const", bufs=1) as cpool, \
             tc.tile_pool(name="xin", bufs=6) as xpool, \
             tc.tile_pool(name="stage", bufs=4) as spool, \
             tc.tile_pool(name="ps", bufs=8, space="PSUM") as ppool:
            wt = cpool.tile([KPAD, FS * NMOV], mybir.dt.bfloat16, name="wt")
            # weight load on the scalar queue so it overlaps the first
            # x-tile load on the sync queue (saves ~1us of startup)
            nc.scalar.dma_start(out=wt[:, :], in_=w[:, :])

            for n in range(n_imgs):
                for t in range(XTILES):
                    xt = xpool.tile([KPAD, HP], mybir.dt.bfloat16,
                                    name="xt", tag="xt")
                    nc.sync.dma_start(out=xt[:, :], in_=x[n, t, :, :])
                    npx = PXT if t < XTILES - 1 else WO - PXT * (XTILES - 1)
                    nv = npx * COUT  # valid moving columns (240 on last tile)
                    last = (n == n_imgs - 1 and t == XTILES - 1)
                    # one wide stage tile per x-tile: 4 y-blocks side by side
                    st = spool.tile([KPAD, 4 * NMOV], mybir.dt.float32,
                                    name="st", tag="st")
                    for bi, y0 in enumerate(YB):
                        ps = ppool.tile([KPAD, NMOV], mybir.dt.float32,
                                        name="ps", tag="ps")
                        for dy in range(FS):
                            nc.tensor.matmul(
                                ps[:, 0:nv],
                                xt[:, y0 + dy: y0 + dy + 128],
                                wt[:, dy * NMOV:dy * NMOV + nv],
                                start=(dy == 0), stop=(dy == FS - 1),
                            )
                        nc.vector.tensor_copy(
                            st[:, bi * NMOV:bi * NMOV + nv], ps[:, 0:nv])
                        if last:
                            # final tile: store each y-block as soon as its
                            # copy lands, shortening the end-of-kernel drain
                            nc.scalar.dma_start(
                                out=out[n, y0:y0 + 128,
                                        PXT * t:PXT * t + npx, :]
                                    .rearrange("y x c -> y (x c)"),
                                in_=st[:, bi * NMOV:bi * NMOV + nv],
                            )
                    if not last:
                        # single merged output DMA for the whole x-tile, on
                        # the scalar HWDGE queue (sync stays free for
                        # prefetch)
                        nc.scalar.dma_start(
                            out=out[n, :, PXT * t:PXT * t + npx, :]
                                .rearrange("(b p) x c -> p b (x c)", p=128),
                            in_=st[:, :].rearrange("p (b j) -> p b j", b=4)
                                        [:, :, 0:nv],
                        )
    _split_excess_waits(nc)
    return nc


def _prep_weights(weights3, weights4, weights4_4, weights6, bias1):
    Wd = _dense_kernel_np(np.asarray(weights3, np.float32),
                          np.asarray(weights4, np.float32),
                          np.asarray(weights4_4, np.float32),
                          np.asarray(weights6, np.float32))
    wb = _build_wband(Wd, bias1)  # [128, 5, 272]
    return np.ascontiguousarray(
        wb.reshape(KPAD, FS * NMOV)).astype(ml_dtypes.bfloat16)


def _prep_inputs(inputs):
    """[B, XTILES, 128, 516] bf16: per-tile transposed windows + ones row."""
    xin = np.asarray(inputs, np.float32).reshape(BATCH, H, FLAT)
    xt_bf = xin.astype(ml_dtypes.bfloat16)  # cast once, contiguous
    xtiles = np.zeros((BATCH, XTILES, KPAD, HP), dtype=ml_dtypes.bfloat16)
    for t in range(XTILES):
        c0 = XSTRIDE * t
        ncols = min(KCOLS, FLAT - c0)
        # [B, ncols, H] <- transpose of [B, H, ncols]
        xtiles[:, t, 0:ncols, 0:H] = xt_bf[:, :, c0:c0 + ncols].swapaxes(1, 2)
    xtiles[:, :, KCOLS, :] = np.float32(1.0)
    return xtiles


def run(inputs, weights3, weights4, weights4_4, weights6, bias1, trace=False):
    from concourse.bass_utils import run_bass_kernel_spmd

    if "nc" not in _CACHE:
        _CACHE["nc"] = _build_nc()
    nc = _CACHE["nc"]

    w_flat = _prep_weights(weights3, weights4, weights4_4, weights6, bias1)
    xtiles = _prep_inputs(inputs)

    in_maps = [
        {"x": xtiles[c * IMGS_PER_CORE:(c + 1) * IMGS_PER_CORE], "w": w_flat}
        for c in range(N_CORES)
    ]
    res = run_bass_kernel_spmd(nc, in_maps, core_ids=list(range(N_CORES)),
                               trace=trace)
    out = np.concatenate([r["out"][:, :HO] for r in res.results], axis=0)
    return out, res


def kernel(inputs, weights3, weights4, weights4_4, weights6, bias1):
    out, _ = run(inputs, weights3, weights4, weights4_4, weights6, bias1)
    return out
